# revision 26
# baseline (speedup 1.0000x reference)
"""Cross-attention (global, batch-flattened K/V) Trainium2 kernel, v7.

Problem: emb [16, 4096, 64]; two cross-attention halves:
  out_l2u = cross(q=emb[:8],  kv=emb[8:])   -> rows 0..7
  out_u2l = cross(q=emb[8:],  kv=emb[:8])   -> rows 8..15
cross(): q/k/v proj (64->512), s = einsum('bnc,nd->bcd', q, kflat),
InstanceNorm over (CH, B*CH) plane per b, softmax over d, ctx = a @ vflat^T,
out = ctx @ Wout.

Sharding: 16 independent (cross, q-batch) instances, 2 per core.
Cores 0-3: q from lower half (kv = upper), cores 4-7: q from upper
(kv = lower). No collectives; weights replicated.

Rank-64 factoring (scores have rank <= 64):
  sT[d,:] for kv batch db = Wk^T @ (emb_db^T @ q)      (m = emb^T q Wq)
  outT = sum_db (wvo_db)^T @ emb_db^T,  wvo_db = (Wv @ aT_db / den) @ Wout
InstanceNorm stats come analytically from m (plane sum via Wk row sums,
sum-of-squares via <m, G2m @ m> with G2m = blockdiag(Wk Wk^T)), so they
are available BEFORE the score plane exists.  Schedule:
  1. m phases chase the streaming input DMA pair by pair, both
     instances back-to-back so the PE runs dense chains (no ping-pong).
  2. Middle is ACT-bound only: score blocks are drained PSUM->SBUF with
     exp(scale*x+bias) FUSED into the drain; inst0's wva matmuls fill
     the PE while inst1's plane drains.
  3. Endgame (wva(1), pwo, outT) is a pure dense PE stretch.
Output stores go out on the gpsimd DMA queue, XBAR transposes early on
sync, so no queue blocks another.
"""

import numpy as np
import concourse.bass as bass
import concourse.mybir as mybir
import concourse.tile as tile
from concourse import bacc
from concourse.bass_utils import run_bass_kernel_spmd

dt = mybir.dt
AF = mybir.ActivationFunctionType
ALU = mybir.AluOpType

B = 8            # batches per half
N = 4096         # sequence length
C = 64           # embedding channels
CH = 512         # num_heads * C
NB = N // 128    # 32 n-blocks
CB = CH // 128   # 4 c-blocks
D = B * CH       # 4096 flattened kv dim
EPS = 1e-5
BF = dt.bfloat16
PLANE = float(CH * D)  # InstanceNorm plane size per instance

_nc = None


def _build():
    nc = bacc.Bacc("TRN2", target_bir_lowering=False, debug=False, num_devices=8)

    embq = nc.declare_dram_parameter("embq", [2, N, C], dt.float32, isOutput=False)
    embkv = nc.declare_dram_parameter("embkv", [B, N, C], dt.float32, isOutput=False)
    Wq_d = nc.declare_dram_parameter("Wq", [C, CH], dt.float32, isOutput=False)
    Wk_d = nc.declare_dram_parameter("Wk", [C, CH], dt.float32, isOutput=False)
    Wv_d = nc.declare_dram_parameter("Wv", [C, CH], dt.float32, isOutput=False)
    Wout_d = nc.declare_dram_parameter("Wout", [CH, C], dt.float32, isOutput=False)
    ident_d = nc.declare_dram_parameter("ident", [128, 128], dt.float32, isOutput=False)
    ones_d = nc.declare_dram_parameter("ones", [128, 128], dt.float32, isOutput=False)
    out_d = nc.declare_dram_parameter("out", [2, C, N], dt.float32, isOutput=True)

    # bf16 kv emb, pair-packed: slot k = kv batches (2k | 2k+1) in cols
    # (0:64 | 64:128); staged in DRAM only for the XBAR-transposed reads.
    # Separate tensors per pair keep the write->transpose deps independent.
    emb_bfs = [nc.dram_tensor(f"emb_bf{k}", [N, 128], BF) for k in range(4)]

    with tile.TileContext(nc) as tc:
        with (
            tc.tile_pool(name="const", bufs=1) as constp,
            tc.tile_pool(name="io", bufs=2) as iop,
            tc.tile_pool(name="res", bufs=1) as resp,
            tc.tile_pool(name="stream", bufs=2) as streamp,
            tc.tile_pool(name="small", bufs=1) as smallp,
            tc.tile_pool(name="ps", bufs=2, space="PSUM") as psp,
        ):
            # ===== input DMA issue (sync queue): q0, kv pair 0 first so
            # the m phase starts as early as possible, then weights, then
            # the rest of the stream. =====
            lt_q0 = iop.tile([128, NB, C], dt.float32, tag="ld", bufs=3)
            nc.sync.dma_start(
                lt_q0[:], embq[0].rearrange("(p nb) c -> p nb c", p=128)
            )
            lt_kv0 = iop.tile([128, NB, C], dt.float32, tag="ld", bufs=3)
            nc.sync.dma_start(
                lt_kv0[:], embkv[0].rearrange("(p nb) c -> p nb c", p=128)
            )
            lt_kv1 = iop.tile([128, NB, C], dt.float32, tag="ld", bufs=3)
            nc.sync.dma_start(
                lt_kv1[:], embkv[1].rearrange("(p nb) c -> p nb c", p=128)
            )

            ident = constp.tile([128, 128], dt.float32, tag="ident")
            nc.sync.dma_start(ident[:], ident_d[:])
            ones_f = iop.tile([128, 128], dt.float32, tag="wst", bufs=4)
            nc.sync.dma_start(ones_f[:], ones_d[:])
            w_st = {}
            for name, wd in (("Wq", Wq_d), ("Wv", Wv_d)):
                wst = iop.tile([C, CH], dt.float32, tag="wst", bufs=4)
                nc.sync.dma_start(wst[:], wd[:])
                w_st[name] = wst
            wk2st = constp.tile([128, CH], dt.float32, tag="wk2st")
            nc.sync.dma_start(wk2st[0:C, :], Wk_d[:])
            nc.sync.dma_start(wk2st[C:128, :], Wk_d[:])
            wost = iop.tile([128, CB, C], dt.float32, tag="wst", bufs=4)
            nc.sync.dma_start(
                wost[:], Wout_d[:].rearrange("(cb p) c -> p cb c", p=128)
            )

            lt_q1 = iop.tile([128, NB, C], dt.float32, tag="ld", bufs=3)
            nc.sync.dma_start(
                lt_q1[:], embq[1].rearrange("(p nb) c -> p nb c", p=128)
            )
            ltq = [lt_q0, lt_q1]
            ltkv = [lt_kv0, lt_kv1]
            for b in range(2, B):
                lt = iop.tile([128, NB, C], dt.float32, tag="ld", bufs=3)
                nc.sync.dma_start(
                    lt[:], embkv[b].rearrange("(p nb) c -> p nb c", p=128)
                )
                ltkv.append(lt)

            # ===== constants / weight prep =====
            ones_r = constp.tile([128, 128], dt.float32r, tag="ones_r")
            nc.vector.tensor_copy(out=ones_r[:], in_=ones_f[:])
            onescol = constp.tile([128, 1], BF, tag="onescol")
            nc.vector.tensor_copy(out=onescol[:], in_=ones_f[:, 0:1])

            Wq_b = constp.tile([C, CH], BF, tag="Wq_bf")
            nc.vector.tensor_copy(out=Wq_b[:], in_=w_st["Wq"][:])
            # Wk duplicated on both partition halves so the sT matmul can
            # read m for the odd batch of a pair at base partition 64
            Wk2_b = constp.tile([128, CH], BF, tag="Wk2_bf")
            nc.vector.tensor_copy(out=Wk2_b[:], in_=wk2st[:])
            # row sums of Wk2 (for plane-sum of s via m)
            wk2s = constp.tile([128, 1], dt.float32, tag="wk2s")
            nc.vector.tensor_reduce(
                out=wk2s[:], in_=Wk2_b[:], axis=mybir.AxisListType.X,
                op=ALU.add,
            )
            # G2m = blockdiag(Wk Wk^T, Wk Wk^T)  (for plane sum-of-squares)
            wk2t_sb = constp.tile([128, CB, 128], BF, tag="wk2t")
            for k in range(CB):
                pbt = psp.tile([128, 512], dt.float32, tag="pp", bufs=2)
                nc.tensor.transpose(
                    pbt[:, 0:128],
                    wk2st[:, k * 128:(k + 1) * 128],
                    ident[:],
                )
                nc.vector.tensor_copy(out=wk2t_sb[:, k, :], in_=pbt[:, 0:128])
            ptg = psp.tile([128, 512], dt.float32, tag="pp", name="ptg")
            for k in range(CB):
                nc.tensor.matmul(
                    ptg[:, 0:128], wk2t_sb[:, k, :], wk2t_sb[:, k, :],
                    start=(k == 0), stop=(k == CB - 1),
                )
            G2m_b = constp.tile([128, 128], BF, tag="G2m_b")
            nc.vector.tensor_copy(out=G2m_b[:], in_=ptg[:, 0:128])
            nc.vector.memset(G2m_b[0:C, C:128], 0.0)
            nc.vector.memset(G2m_b[C:128, 0:C], 0.0)

            # WvT [128(ch sub), CB, 65] via 4 fp32 PE transposes; the 65th
            # column is ones so the wva matmuls also produce the softmax
            # denominator partials.
            WvT_b = constp.tile([128, CB, C + 1], BF, tag="WvT_bf")
            ptw = psp.tile([128, 512], dt.float32, tag="pp")
            for k in range(CB):
                nc.tensor.transpose(
                    ptw[:, k * 128:k * 128 + C],
                    w_st["Wv"][:, k * 128:(k + 1) * 128],
                    ident[0:C, 0:C],
                )
            for k in range(CB):
                nc.vector.tensor_copy(
                    out=WvT_b[:, k, 0:C], in_=ptw[:, k * 128:k * 128 + C]
                )
                nc.vector.tensor_copy(
                    out=WvT_b[:, k, C:C + 1], in_=onescol[:]
                )

            Wout_b = constp.tile([128, CB, C], BF, tag="Wout_bf")
            nc.vector.tensor_copy(out=Wout_b[:], in_=wost[:])

            # ===== streamed casts + staging =====
            ebq_tiles = []
            for inst in range(2):
                ebq = iop.tile([128, NB, C], BF, tag="ebq", bufs=2)
                nc.vector.tensor_copy(out=ebq[:], in_=ltq[inst][:])
                ebq_tiles.append(ebq)

            lb_tiles = []
            for pair in range(4):
                lb = iop.tile([128, NB, 128], BF, tag="ldb", bufs=4)
                nc.vector.tensor_copy(out=lb[:, :, 0:C], in_=ltkv[2 * pair][:])
                nc.scalar.activation(lb[:, :, C:128], ltkv[2 * pair + 1][:],
                                     AF.Copy)
                lb_tiles.append(lb)

            # Staging gate: a 1-element DMA into each emb_bf tensor whose
            # source depends on the LAST pair's cast.  The 1MB staging
            # writes (WAW on that element) and the XBAR transposes are
            # thereby deferred until the input loads have drained, so they
            # don't steal HBM bandwidth from the critical input stream.
            gate_sb = smallp.tile([1, 4], BF, tag="gate")
            nc.vector.tensor_tensor(
                out=gate_sb[:], in0=lb_tiles[3][0:1, 0, 0:4],
                in1=lb_tiles[3][0:1, 0, 0:4], op=ALU.bypass,
            )
            etp_tiles = []
            for pair in range(4):
                nc.sync.dma_start(
                    emb_bfs[pair][0:1, 0:1], gate_sb[0:1, pair:pair + 1]
                )
                nc.scalar.dma_start(
                    emb_bfs[pair].rearrange("(p nb) c -> p nb c", p=128),
                    lb_tiles[pair][:],
                )
                etp = streamp.tile([128, N], BF, tag="etp", bufs=4)
                nc.sync.dma_start_transpose(etp[:], emb_bfs[pair][:])
                etp_tiles.append(etp)

            # ===== persistent small tensors =====
            ssum = smallp.tile([128, 2, 4], dt.float32, tag="ssum")
            ssq = smallp.tile([128, 2, 4], dt.float32, tag="ssq")
            gmtmp = smallp.tile([128, CH], dt.float32, tag="gmtmp")
            mrs = smallp.tile([128, 1], dt.float32, tag="mrs", bufs=2)
            stats2 = smallp.tile([128, 8, 2], dt.float32, tag="stats2")
            invden2 = smallp.tile([128, 2, CB], dt.float32, tag="invden2")
            den_acc = [
                smallp.tile([128, CB, 1], dt.float32, tag="denacc", bufs=2,
                            name=f"den{i}")
                for i in range(2)
            ]

            m_tiles = {}

            def m_phase_pe(inst, pair):
                """E2T = emb_q^T @ [emb_2p | emb_2p+1]  ([64, 128])."""
                pE = psp.tile([128, 512], dt.float32, tag="pp",
                              name=f"pE{inst}_{pair}")
                lb = lb_tiles[pair]
                ebq = ebq_tiles[inst]
                for nb in range(NB):
                    nc.tensor.matmul(
                        pE[0:C, 0:128],
                        ebq[:, nb, :],
                        lb[:, nb, :],
                        start=(nb == 0),
                        stop=(nb == NB - 1),
                    )
                e2t = streamp.tile([C, 128], BF, tag="e2t")
                nc.vector.tensor_copy(out=e2t[:], in_=pE[0:C, 0:128])
                return e2t

            def m_phase_m(inst, pair, e2t):
                """m = E2T^T @ Wq ([128, CH]); plane-sum partials."""
                pm = psp.tile([128, 512], dt.float32, tag="pp",
                              name=f"pm{inst}_{pair}")
                nc.tensor.matmul(
                    pm[:], e2t[:], Wq_b[:], start=True, stop=True
                )
                m_sb = streamp.tile([128, CH], BF, tag="msb", bufs=8)
                nc.vector.tensor_copy(out=m_sb[:], in_=pm[:])
                m_tiles[(inst, pair)] = m_sb
                nc.vector.tensor_reduce(
                    out=mrs[:], in_=m_sb[:], axis=mybir.AxisListType.X,
                    op=ALU.add,
                )
                nc.vector.tensor_tensor(
                    out=ssum[:, inst, pair:pair + 1], in0=mrs[:],
                    in1=wk2s[:], op=ALU.mult,
                )

            def pair_ssq(inst, pair):
                """plane sum-of-squares partial: <m, G2m @ m>."""
                m_sb = m_tiles[(inst, pair)]
                pgm = psp.tile([128, 512], dt.float32, tag="pp",
                               name=f"pgm{inst}_{pair}")
                nc.tensor.matmul(
                    pgm[:], G2m_b[:], m_sb[:], start=True, stop=True
                )
                nc.vector.tensor_tensor(
                    out=gmtmp[:], in0=m_sb[:], in1=pgm[:], op=ALU.mult,
                )
                nc.vector.tensor_reduce(
                    out=ssq[:, inst, pair:pair + 1], in_=gmtmp[:],
                    axis=mybir.AxisListType.X, op=ALU.add,
                )

            def stats_prep_both():
                """InstanceNorm stats for BOTH instances in one pass.
                Everything on DVE except a single [128, 2] Sqrt, so the
                ACT engine sees at most one table switch before the exp
                stream (table thrash mid-stream costs 1.3us per switch).
                stats2 layout: [128, stat, inst]."""
                red = smallp.tile([128, 4], dt.float32, tag="red")
                for inst in range(2):
                    nc.vector.tensor_reduce(
                        out=red[:, 2 * inst:2 * inst + 1],
                        in_=ssum[:, inst, :],
                        axis=mybir.AxisListType.X, op=ALU.add,
                    )
                    nc.vector.tensor_reduce(
                        out=red[:, 2 * inst + 1:2 * inst + 2],
                        in_=ssq[:, inst, :],
                        axis=mybir.AxisListType.X, op=ALU.add,
                    )
                red_r = smallp.tile([128, 4], dt.float32r, tag="red_r")
                nc.vector.tensor_copy(out=red_r[:], in_=red[:])
                ptr = psp.tile([128, 512], dt.float32, tag="pp", name="ptr")
                nc.tensor.matmul(
                    ptr[:, 0:4], ones_r[:], red_r[:], start=True, stop=True
                )
                # ptr cols = (sum0, sumsq0, sum1, sumsq1) -> stat-major
                nc.vector.tensor_scalar_mul(
                    stats2[:, 0:2, :],
                    ptr[:, 0:4].rearrange("p (i k) -> p k i", i=2),
                    1.0 / PLANE,
                )
                mu = stats2[:, 0, :]
                ex2 = stats2[:, 1, :]
                musq = stats2[:, 2, :]
                var = stats2[:, 3, :]
                std = stats2[:, 4, :]
                rstd = stats2[:, 5, :]
                nmr = stats2[:, 6, :]
                nc.vector.tensor_tensor(out=musq, in0=mu, in1=mu, op=ALU.mult)
                nc.vector.tensor_tensor(out=var, in0=ex2, in1=musq,
                                        op=ALU.subtract)
                nc.vector.tensor_scalar_add(var, var, EPS)
                nc.scalar.activation(std, var, AF.Sqrt, bias=0.0)
                nc.vector.reciprocal(rstd, std)
                nc.vector.tensor_tensor(out=nmr, in0=mu, in1=rstd,
                                        op=ALU.mult)
                nc.vector.tensor_scalar_mul(nmr, nmr, -1.0)

            sT_tiles = [
                resp.tile([128, NB, CH], BF, tag="sT", bufs=2,
                          name=f"sT{i}")
                for i in range(2)
            ]

            def sT_fill(inst, db):
                """sT[d, c] = exp(rstd * Wk^T m - mu*rstd): two 2-bank PSUM
                chunks, each drained [128, 1024] by ACT with fused exp."""
                half = (db % 2) * C
                m_sb = m_tiles[(inst, db // 2)]
                sT = sT_tiles[inst]
                for c in range(2):
                    ps2 = psp.tile([128, 2, 512], dt.float32, tag="ps2",
                                   bufs=3, name=f"ps2_{inst}_{db}_{c}")
                    for d2 in range(2):
                        dc = 2 * c + d2
                        nc.tensor.matmul(
                            ps2[:, d2, :],
                            Wk2_b[half:half + C, dc * 128:(dc + 1) * 128],
                            m_sb[half:half + C, :],
                            start=True,
                            stop=True,
                        )
                    kb = db * CB + 2 * c
                    nc.scalar.activation(
                        sT[:, kb:kb + 2, :], ps2[:], AF.Exp,
                        bias=stats2[:, 6, inst:inst + 1],
                        scale=stats2[:, 5, inst:inst + 1],
                    )

            wva_tiles = [
                streamp.tile([128, B, CB, C], BF, tag="wva8", bufs=2,
                             name=f"wva{i}")
                for i in range(2)
            ]

            def wva(inst, db):
                """wva_db = (a_db Wv^T | den partials): 16 PE matmuls off
                the exp'd sT block, drained by a single DVE copy."""
                sT = sT_tiles[inst]
                W65 = C + 1
                pwa = psp.tile([128, 512], dt.float32, tag="pp",
                               name=f"pwa{inst}_{db}")
                for chb in range(CB):
                    for j in range(CB):
                        nc.tensor.matmul(
                            pwa[:, chb * 128:chb * 128 + W65],
                            sT[:, db * CB + j, chb * 128:(chb + 1) * 128],
                            WvT_b[:, j, :],
                            start=(j == 0),
                            stop=(j == CB - 1),
                        )
                wva_all = wva_tiles[inst]
                pview = pwa[:].rearrange("p (a b) -> p a b", b=128)
                nc.vector.tensor_copy(
                    out=wva_all[:, db, :, :], in_=pview[:, :, 0:C],
                )
                dcol = pview[:, :, C:C + 1]
                if db == 0:
                    nc.vector.tensor_copy(out=den_acc[inst][:], in_=dcol)
                else:
                    nc.vector.tensor_tensor(
                        out=den_acc[inst][:], in0=den_acc[inst][:],
                        in1=dcol, op=ALU.add,
                    )

            def ctx_out(inst):
                """outT = sum_db (wvo_db)^T @ emb_db^T with
                wvo_db = wva_db @ (Wout / den)."""
                wva_all = wva_tiles[inst]
                nc.vector.reciprocal(
                    invden2[:, inst, :], den_acc[inst][:, :, 0]
                )
                wout_s = streamp.tile([128, CB, C], BF, tag="wout_s",
                                      bufs=2)
                for k in range(CB):
                    nc.vector.tensor_scalar_mul(
                        wout_s[:, k, :], Wout_b[:, k, :],
                        invden2[:, inst, k:k + 1],
                    )
                pwo = psp.tile([128, 512], dt.float32, tag="pp",
                               name=f"pwo{inst}")
                for db in range(B):
                    for chb in range(CB):
                        nc.tensor.matmul(
                            pwo[0:C, db * C:(db + 1) * C],
                            wva_all[:, db, chb, :],
                            wout_s[:, chb, :],
                            start=(chb == 0),
                            stop=(chb == CB - 1),
                        )
                # pair-stack wvo: [128(2 batches' c'), 4(pair), 64]
                wvo2 = streamp.tile([128, CB, C], BF, tag="wvo2", bufs=2)
                for db in range(B):
                    nc.vector.tensor_copy(
                        out=wvo2[(db % 2) * C:(db % 2 + 1) * C, db // 2, :],
                        in_=pwo[0:C, db * C:(db + 1) * C],
                    )
                # outT[c, n] = sum_pairs wvo_pair^T @ embT_pair
                for g in range(8):
                    pout = psp.tile([128, 512], dt.float32, tag="pp",
                                    name=f"pout{inst}_{g}")
                    for k in range(4):
                        nc.tensor.matmul(
                            pout[0:C, :],
                            wvo2[:, k, :],
                            etp_tiles[k][:, g * 512:(g + 1) * 512],
                            start=(k == 0),
                            stop=(k == 3),
                        )
                    ot = streamp.tile([C, 512], dt.float32, tag="ot",
                                      bufs=2)
                    # DVE-only drains: any ACT op here would clock-gate
                    # the PE (K=4) during the pure-matmul endgame.
                    nc.vector.tensor_copy(out=ot[:], in_=pout[0:C, :])
                    nc.sync.dma_start(
                        out_d[inst, :, g * 512:(g + 1) * 512], ot[:]
                    )

            # ================= schedule =================
            # m phases chase the input stream: dense PE chains per pair
            # (both instances), DVE work trails without blocking the PE.
            for pair in range(4):
                e2ts = [m_phase_pe(inst, pair) for inst in range(2)]
                for inst in range(2):
                    m_phase_m(inst, pair, e2ts[inst])
            for inst in range(2):
                for pair in range(4):
                    pair_ssq(inst, pair)
            stats_prep_both()

            # middle: inst0's plane fills+drains, then inst1's, with
            # inst0's wva matmuls filling the PE while inst1 drains.
            for db in range(B):
                sT_fill(0, db)
            for db in range(B):
                sT_fill(1, db)
                wva(0, db)
            for db in range(B):
                wva(1, db)

            ctx_out(0)
            ctx_out(1)

    nc.compile()
    return nc


def _get_nc():
    global _nc
    if _nc is None:
        _nc = _build()
    return _nc


def kernel(emb, Wq, Wk, Wv, Wout):
    emb = np.ascontiguousarray(emb, dtype=np.float32)
    Wq = np.ascontiguousarray(Wq, dtype=np.float32)
    Wk = np.ascontiguousarray(Wk, dtype=np.float32)
    Wv = np.ascontiguousarray(Wv, dtype=np.float32)
    Wout = np.ascontiguousarray(Wout, dtype=np.float32)
    emb_l, emb_u = emb[:B], emb[B:]
    ident = np.eye(128, dtype=np.float32)
    ones = np.ones((128, 128), dtype=np.float32)

    in_maps = []
    for core in range(8):
        if core < 4:
            qb, kvb = emb_l[2 * core:2 * core + 2], emb_u
        else:
            j = core - 4
            qb, kvb = emb_u[2 * j:2 * j + 2], emb_l
        in_maps.append({
            "embq": np.ascontiguousarray(qb), "embkv": np.ascontiguousarray(kvb),
            "Wq": Wq, "Wk": Wk, "Wv": Wv, "Wout": Wout, "ident": ident,
            "ones": ones,
        })

    res = run_bass_kernel_spmd(_get_nc(), in_maps, list(range(8))).results

    out = np.empty((2 * B, N, C), np.float32)
    for core in range(8):
        o = res[core]["out"].transpose(0, 2, 1)  # [2, C, N] -> [2, N, C]
        if core < 4:
            out[2 * core:2 * core + 2] = o
        else:
            j = core - 4
            out[B + 2 * j:B + 2 * j + 2] = o
    return out


# revision 30
# speedup vs baseline: 1.0127x; 1.0127x over previous
"""Cross-attention (global, batch-flattened K/V) Trainium2 kernel, v7.

Problem: emb [16, 4096, 64]; two cross-attention halves:
  out_l2u = cross(q=emb[:8],  kv=emb[8:])   -> rows 0..7
  out_u2l = cross(q=emb[8:],  kv=emb[:8])   -> rows 8..15
cross(): q/k/v proj (64->512), s = einsum('bnc,nd->bcd', q, kflat),
InstanceNorm over (CH, B*CH) plane per b, softmax over d, ctx = a @ vflat^T,
out = ctx @ Wout.

Sharding: 16 independent (cross, q-batch) instances, 2 per core.
Cores 0-3: q from lower half (kv = upper), cores 4-7: q from upper
(kv = lower). No collectives; weights replicated.

Rank-64 factoring (scores have rank <= 64):
  sT[d,:] for kv batch db = Wk^T @ (emb_db^T @ q)      (m = emb^T q Wq)
  outT = sum_db (wvo_db)^T @ emb_db^T,  wvo_db = (Wv @ aT_db / den) @ Wout
InstanceNorm stats come analytically from m (plane sum via Wk row sums,
sum-of-squares via <m, G2m @ m> with G2m = blockdiag(Wk Wk^T)), so they
are available BEFORE the score plane exists.  Schedule:
  1. m phases chase the streaming input DMA pair by pair, both
     instances back-to-back so the PE runs dense chains (no ping-pong).
  2. Middle is ACT-bound only: score blocks are drained PSUM->SBUF with
     exp(scale*x+bias) FUSED into the drain; inst0's wva matmuls fill
     the PE while inst1's plane drains.
  3. Endgame (wva(1), pwo, outT) is a pure dense PE stretch.
Output stores go out on the gpsimd DMA queue, XBAR transposes early on
sync, so no queue blocks another.
"""

import numpy as np
import concourse.bass as bass
import concourse.mybir as mybir
import concourse.tile as tile
from concourse import bacc
from concourse.bass_utils import run_bass_kernel_spmd

dt = mybir.dt
AF = mybir.ActivationFunctionType
ALU = mybir.AluOpType

B = 8            # batches per half
N = 4096         # sequence length
C = 64           # embedding channels
CH = 512         # num_heads * C
NB = N // 128    # 32 n-blocks
CB = CH // 128   # 4 c-blocks
D = B * CH       # 4096 flattened kv dim
EPS = 1e-5
BF = dt.bfloat16
PLANE = float(CH * D)  # InstanceNorm plane size per instance

_nc = None


def _build():
    nc = bacc.Bacc("TRN2", target_bir_lowering=False, debug=False, num_devices=8)

    embq = nc.declare_dram_parameter("embq", [2, N, C], dt.float32, isOutput=False)
    embkv = nc.declare_dram_parameter("embkv", [B, N, C], dt.float32, isOutput=False)
    Wq_d = nc.declare_dram_parameter("Wq", [C, CH], dt.float32, isOutput=False)
    Wk_d = nc.declare_dram_parameter("Wk", [C, CH], dt.float32, isOutput=False)
    Wv_d = nc.declare_dram_parameter("Wv", [C, CH], dt.float32, isOutput=False)
    Wout_d = nc.declare_dram_parameter("Wout", [CH, C], dt.float32, isOutput=False)
    ident_d = nc.declare_dram_parameter("ident", [128, 128], dt.float32, isOutput=False)
    ones_d = nc.declare_dram_parameter("ones", [128, 128], dt.float32, isOutput=False)
    out_d = nc.declare_dram_parameter("out", [2, C, N], dt.float32, isOutput=True)

    # bf16 kv emb, pair-packed: slot k = kv batches (2k | 2k+1) in cols
    # (0:64 | 64:128); staged in DRAM only for the XBAR-transposed reads.
    # Separate tensors per pair keep the write->transpose deps independent.
    emb_bfs = [nc.dram_tensor(f"emb_bf{k}", [N, 128], BF) for k in range(4)]

    with tile.TileContext(nc) as tc:
        with (
            tc.tile_pool(name="const", bufs=1) as constp,
            tc.tile_pool(name="io", bufs=2) as iop,
            tc.tile_pool(name="res", bufs=1) as resp,
            tc.tile_pool(name="stream", bufs=2) as streamp,
            tc.tile_pool(name="small", bufs=1) as smallp,
            tc.tile_pool(name="ps", bufs=2, space="PSUM") as psp,
        ):
            # ===== input DMA issue (sync queue): q0, kv pair 0 first so
            # the m phase starts as early as possible, then weights, then
            # the rest of the stream. =====
            lt_q0 = iop.tile([128, NB, C], dt.float32, tag="ld", bufs=3)
            nc.sync.dma_start(
                lt_q0[:], embq[0].rearrange("(p nb) c -> p nb c", p=128)
            )
            lt_kv0 = iop.tile([128, NB, C], dt.float32, tag="ld", bufs=3)
            nc.scalar.dma_start(
                lt_kv0[:], embkv[0].rearrange("(p nb) c -> p nb c", p=128)
            )
            lt_kv1 = iop.tile([128, NB, C], dt.float32, tag="ld", bufs=3)
            nc.sync.dma_start(
                lt_kv1[:], embkv[1].rearrange("(p nb) c -> p nb c", p=128)
            )

            ident = constp.tile([128, 128], dt.float32, tag="ident")
            nc.sync.dma_start(ident[:], ident_d[:])
            ones_f = iop.tile([128, 128], dt.float32, tag="wst", bufs=4)
            nc.sync.dma_start(ones_f[:], ones_d[:])
            w_st = {}
            for name, wd in (("Wq", Wq_d), ("Wv", Wv_d)):
                wst = iop.tile([C, CH], dt.float32, tag="wst", bufs=4)
                nc.sync.dma_start(wst[:], wd[:])
                w_st[name] = wst
            wk2st = constp.tile([128, CH], dt.float32, tag="wk2st")
            nc.sync.dma_start(wk2st[0:C, :], Wk_d[:])
            nc.sync.dma_start(wk2st[C:128, :], Wk_d[:])
            wost = iop.tile([128, CB, C], dt.float32, tag="wst", bufs=4)
            nc.sync.dma_start(
                wost[:], Wout_d[:].rearrange("(cb p) c -> p cb c", p=128)
            )

            lt_q1 = iop.tile([128, NB, C], dt.float32, tag="ld", bufs=3)
            nc.scalar.dma_start(
                lt_q1[:], embq[1].rearrange("(p nb) c -> p nb c", p=128)
            )
            ltq = [lt_q0, lt_q1]
            ltkv = [lt_kv0, lt_kv1]
            for b in range(2, B):
                lt = iop.tile([128, NB, C], dt.float32, tag="ld", bufs=3)
                eng = nc.sync if b % 2 == 0 else nc.scalar
                eng.dma_start(
                    lt[:], embkv[b].rearrange("(p nb) c -> p nb c", p=128)
                )
                ltkv.append(lt)

            # ===== constants / weight prep =====
            ones_r = constp.tile([128, 128], dt.float32r, tag="ones_r")
            nc.vector.tensor_copy(out=ones_r[:], in_=ones_f[:])
            onescol = constp.tile([128, 1], BF, tag="onescol")
            nc.vector.tensor_copy(out=onescol[:], in_=ones_f[:, 0:1])

            Wq_b = constp.tile([C, CH], BF, tag="Wq_bf")
            nc.vector.tensor_copy(out=Wq_b[:], in_=w_st["Wq"][:])
            # Wk duplicated on both partition halves so the sT matmul can
            # read m for the odd batch of a pair at base partition 64
            Wk2_b = constp.tile([128, CH], BF, tag="Wk2_bf")
            nc.vector.tensor_copy(out=Wk2_b[:], in_=wk2st[:])
            # row sums of Wk2 (for plane-sum of s via m)
            wk2s = constp.tile([128, 1], dt.float32, tag="wk2s")
            nc.vector.tensor_reduce(
                out=wk2s[:], in_=Wk2_b[:], axis=mybir.AxisListType.X,
                op=ALU.add,
            )
            # G2m = blockdiag(Wk Wk^T, Wk Wk^T)  (for plane sum-of-squares)
            wk2t_sb = constp.tile([128, CB, 128], BF, tag="wk2t")
            for k in range(CB):
                pbt = psp.tile([128, 512], dt.float32, tag="pp", bufs=2)
                nc.tensor.transpose(
                    pbt[:, 0:128],
                    wk2st[:, k * 128:(k + 1) * 128],
                    ident[:],
                )
                nc.vector.tensor_copy(out=wk2t_sb[:, k, :], in_=pbt[:, 0:128])
            ptg = psp.tile([128, 512], dt.float32, tag="pp", name="ptg")
            for k in range(CB):
                nc.tensor.matmul(
                    ptg[:, 0:128], wk2t_sb[:, k, :], wk2t_sb[:, k, :],
                    start=(k == 0), stop=(k == CB - 1),
                )
            G2m_b = constp.tile([128, 128], BF, tag="G2m_b")
            nc.vector.tensor_copy(out=G2m_b[:], in_=ptg[:, 0:128])
            nc.vector.memset(G2m_b[0:C, C:128], 0.0)
            nc.vector.memset(G2m_b[C:128, 0:C], 0.0)

            # WvT [128(ch sub), CB, 65] via 4 fp32 PE transposes; the 65th
            # column is ones so the wva matmuls also produce the softmax
            # denominator partials.
            WvT_b = constp.tile([128, CB, C + 1], BF, tag="WvT_bf")
            ptw = psp.tile([128, 512], dt.float32, tag="pp")
            for k in range(CB):
                nc.tensor.transpose(
                    ptw[:, k * 128:k * 128 + C],
                    w_st["Wv"][:, k * 128:(k + 1) * 128],
                    ident[0:C, 0:C],
                )
            for k in range(CB):
                nc.vector.tensor_copy(
                    out=WvT_b[:, k, 0:C], in_=ptw[:, k * 128:k * 128 + C]
                )
                nc.vector.tensor_copy(
                    out=WvT_b[:, k, C:C + 1], in_=onescol[:]
                )

            Wout_b = constp.tile([128, CB, C], BF, tag="Wout_bf")
            nc.vector.tensor_copy(out=Wout_b[:], in_=wost[:])

            # ===== streamed casts + staging =====
            ebq_tiles = []
            for inst in range(2):
                ebq = iop.tile([128, NB, C], BF, tag="ebq", bufs=2)
                nc.vector.tensor_copy(out=ebq[:], in_=ltq[inst][:])
                ebq_tiles.append(ebq)

            lb_tiles = []
            for pair in range(4):
                lb = iop.tile([128, NB, 128], BF, tag="ldb", bufs=4)
                nc.vector.tensor_copy(out=lb[:, :, 0:C], in_=ltkv[2 * pair][:])
                nc.scalar.activation(lb[:, :, C:128], ltkv[2 * pair + 1][:],
                                     AF.Copy)
                lb_tiles.append(lb)

            # Staging gate: a 1-element DMA into each emb_bf tensor whose
            # source depends on the LAST pair's cast.  The 1MB staging
            # writes (WAW on that element) and the XBAR transposes are
            # thereby deferred until the input loads have drained, so they
            # don't steal HBM bandwidth from the critical input stream.
            # Everything lives on the sync ring (emitted later, after the
            # stats) so the ACT ring never stalls behind a staging issue.
            gate_sb = smallp.tile([1, 4], BF, tag="gate")
            nc.vector.tensor_tensor(
                out=gate_sb[:], in0=lb_tiles[3][0:1, 0, 0:4],
                in1=lb_tiles[3][0:1, 0, 0:4], op=ALU.bypass,
            )
            etp_tiles = []

            def stage_etp():
                for pair in range(4):
                    nc.sync.dma_start(
                        emb_bfs[pair][0:1, 0:1], gate_sb[0:1, pair:pair + 1]
                    )
                for pair in range(4):
                    nc.sync.dma_start(
                        emb_bfs[pair].rearrange("(p nb) c -> p nb c", p=128),
                        lb_tiles[pair][:],
                    )
                for pair in range(4):
                    etp = streamp.tile([128, N], BF, tag="etp", bufs=4)
                    nc.sync.dma_start_transpose(etp[:], emb_bfs[pair][:])
                    etp_tiles.append(etp)

            # ===== persistent small tensors =====
            ssum = smallp.tile([128, 2, 4], dt.float32, tag="ssum")
            ssq = smallp.tile([128, 2, 4], dt.float32, tag="ssq")
            gmtmp = smallp.tile([128, CH], dt.float32, tag="gmtmp")
            mrs = smallp.tile([128, 1], dt.float32, tag="mrs", bufs=2)
            stats2 = smallp.tile([128, 8, 2], dt.float32, tag="stats2")
            invden2 = smallp.tile([128, 2, CB], dt.float32, tag="invden2")
            den_acc = [
                smallp.tile([128, CB, 1], dt.float32, tag="denacc", bufs=2,
                            name=f"den{i}")
                for i in range(2)
            ]

            m_tiles = {}

            def m_phase_pe(inst, pair):
                """E2T = emb_q^T @ [emb_2p | emb_2p+1]  ([64, 128])."""
                pE = psp.tile([128, 512], dt.float32, tag="pp",
                              name=f"pE{inst}_{pair}")
                lb = lb_tiles[pair]
                ebq = ebq_tiles[inst]
                for nb in range(NB):
                    nc.tensor.matmul(
                        pE[0:C, 0:128],
                        ebq[:, nb, :],
                        lb[:, nb, :],
                        start=(nb == 0),
                        stop=(nb == NB - 1),
                    )
                e2t = streamp.tile([C, 128], BF, tag="e2t")
                nc.vector.tensor_copy(out=e2t[:], in_=pE[0:C, 0:128])
                return e2t

            def m_phase_m(inst, pair, e2t):
                """m = E2T^T @ Wq ([128, CH]); plane-sum partials."""
                pm = psp.tile([128, 512], dt.float32, tag="pp",
                              name=f"pm{inst}_{pair}")
                nc.tensor.matmul(
                    pm[:], e2t[:], Wq_b[:], start=True, stop=True
                )
                m_sb = streamp.tile([128, CH], BF, tag="msb", bufs=8)
                nc.vector.tensor_copy(out=m_sb[:], in_=pm[:])
                m_tiles[(inst, pair)] = m_sb
                nc.vector.tensor_reduce(
                    out=mrs[:], in_=m_sb[:], axis=mybir.AxisListType.X,
                    op=ALU.add,
                )
                nc.vector.tensor_tensor(
                    out=ssum[:, inst, pair:pair + 1], in0=mrs[:],
                    in1=wk2s[:], op=ALU.mult,
                )

            def pair_ssq(inst, pair):
                """plane sum-of-squares partial: <m, G2m @ m>."""
                m_sb = m_tiles[(inst, pair)]
                pgm = psp.tile([128, 512], dt.float32, tag="pp",
                               name=f"pgm{inst}_{pair}")
                nc.tensor.matmul(
                    pgm[:], G2m_b[:], m_sb[:], start=True, stop=True
                )
                nc.vector.tensor_tensor(
                    out=gmtmp[:], in0=m_sb[:], in1=pgm[:], op=ALU.mult,
                )
                nc.vector.tensor_reduce(
                    out=ssq[:, inst, pair:pair + 1], in_=gmtmp[:],
                    axis=mybir.AxisListType.X, op=ALU.add,
                )

            def stats_prep_both():
                """InstanceNorm stats for BOTH instances in one pass.
                Everything on DVE except a single [128, 2] Sqrt, so the
                ACT engine sees at most one table switch before the exp
                stream (table thrash mid-stream costs 1.3us per switch).
                stats2 layout: [128, stat, inst]."""
                red = smallp.tile([128, 4], dt.float32, tag="red")
                for inst in range(2):
                    nc.vector.tensor_reduce(
                        out=red[:, 2 * inst:2 * inst + 1],
                        in_=ssum[:, inst, :],
                        axis=mybir.AxisListType.X, op=ALU.add,
                    )
                    nc.vector.tensor_reduce(
                        out=red[:, 2 * inst + 1:2 * inst + 2],
                        in_=ssq[:, inst, :],
                        axis=mybir.AxisListType.X, op=ALU.add,
                    )
                red_r = smallp.tile([128, 4], dt.float32r, tag="red_r")
                nc.vector.tensor_copy(out=red_r[:], in_=red[:])
                ptr = psp.tile([128, 512], dt.float32, tag="pp", name="ptr")
                nc.tensor.matmul(
                    ptr[:, 0:4], ones_r[:], red_r[:], start=True, stop=True
                )
                # ptr cols = (sum0, sumsq0, sum1, sumsq1) -> stat-major
                nc.vector.tensor_scalar_mul(
                    stats2[:, 0:2, :],
                    ptr[:, 0:4].rearrange("p (i k) -> p k i", i=2),
                    1.0 / PLANE,
                )
                mu = stats2[:, 0, :]
                ex2 = stats2[:, 1, :]
                musq = stats2[:, 2, :]
                var = stats2[:, 3, :]
                std = stats2[:, 4, :]
                rstd = stats2[:, 5, :]
                nmr = stats2[:, 6, :]
                nc.vector.tensor_tensor(out=musq, in0=mu, in1=mu, op=ALU.mult)
                nc.vector.tensor_tensor(out=var, in0=ex2, in1=musq,
                                        op=ALU.subtract)
                nc.vector.tensor_scalar_add(var, var, EPS)
                nc.scalar.activation(std, var, AF.Sqrt, bias=0.0)
                nc.vector.reciprocal(rstd, std)
                nc.vector.tensor_tensor(out=nmr, in0=mu, in1=rstd,
                                        op=ALU.mult)
                nc.vector.tensor_scalar_mul(nmr, nmr, -1.0)

            sT_tiles = [
                resp.tile([128, NB, CH], BF, tag="sT", bufs=2,
                          name=f"sT{i}")
                for i in range(2)
            ]

            def sT_fill(inst, db):
                """sT[d, c] = exp(rstd * Wk^T m - mu*rstd): two 2-bank PSUM
                chunks, each drained [128, 1024] by ACT with fused exp."""
                half = (db % 2) * C
                m_sb = m_tiles[(inst, db // 2)]
                sT = sT_tiles[inst]
                for c in range(2):
                    ps2 = psp.tile([128, 2, 512], dt.float32, tag="ps2",
                                   bufs=3, name=f"ps2_{inst}_{db}_{c}")
                    for d2 in range(2):
                        dc = 2 * c + d2
                        nc.tensor.matmul(
                            ps2[:, d2, :],
                            Wk2_b[half:half + C, dc * 128:(dc + 1) * 128],
                            m_sb[half:half + C, :],
                            start=True,
                            stop=True,
                        )
                    kb = db * CB + 2 * c
                    nc.scalar.activation(
                        sT[:, kb:kb + 2, :], ps2[:], AF.Exp,
                        bias=stats2[:, 6, inst:inst + 1],
                        scale=stats2[:, 5, inst:inst + 1],
                    )

            wva_tiles = [
                streamp.tile([128, B, CB, C], BF, tag="wva8", bufs=2,
                             name=f"wva{i}")
                for i in range(2)
            ]

            def wva(inst, db):
                """wva_db = (a_db Wv^T | den partials): 16 PE matmuls off
                the exp'd sT block, drained by a single DVE copy."""
                sT = sT_tiles[inst]
                W65 = C + 1
                pwa = psp.tile([128, 512], dt.float32, tag="pp",
                               name=f"pwa{inst}_{db}")
                for chb in range(CB):
                    for j in range(CB):
                        nc.tensor.matmul(
                            pwa[:, chb * 128:chb * 128 + W65],
                            sT[:, db * CB + j, chb * 128:(chb + 1) * 128],
                            WvT_b[:, j, :],
                            start=(j == 0),
                            stop=(j == CB - 1),
                        )
                wva_all = wva_tiles[inst]
                pview = pwa[:].rearrange("p (a b) -> p a b", b=128)
                nc.vector.tensor_copy(
                    out=wva_all[:, db, :, :], in_=pview[:, :, 0:C],
                )
                dcol = pview[:, :, C:C + 1]
                if db == 0:
                    nc.vector.tensor_copy(out=den_acc[inst][:], in_=dcol)
                else:
                    nc.vector.tensor_tensor(
                        out=den_acc[inst][:], in0=den_acc[inst][:],
                        in1=dcol, op=ALU.add,
                    )

            def ctx_out(inst):
                """outT = sum_db (wvo_db)^T @ emb_db^T with
                wvo_db = wva_db @ (Wout / den)."""
                wva_all = wva_tiles[inst]
                nc.vector.reciprocal(
                    invden2[:, inst, :], den_acc[inst][:, :, 0]
                )
                wout_s = streamp.tile([128, CB, C], BF, tag="wout_s",
                                      bufs=2)
                for k in range(CB):
                    nc.vector.tensor_scalar_mul(
                        wout_s[:, k, :], Wout_b[:, k, :],
                        invden2[:, inst, k:k + 1],
                    )
                pwo = psp.tile([128, 512], dt.float32, tag="pp",
                               name=f"pwo{inst}")
                for db in range(B):
                    for chb in range(CB):
                        nc.tensor.matmul(
                            pwo[0:C, db * C:(db + 1) * C],
                            wva_all[:, db, chb, :],
                            wout_s[:, chb, :],
                            start=(chb == 0),
                            stop=(chb == CB - 1),
                        )
                # pair-stack wvo: [128(2 batches' c'), 4(pair), 64]
                wvo2 = streamp.tile([128, CB, C], BF, tag="wvo2", bufs=2)
                for db in range(B):
                    nc.vector.tensor_copy(
                        out=wvo2[(db % 2) * C:(db % 2 + 1) * C, db // 2, :],
                        in_=pwo[0:C, db * C:(db + 1) * C],
                    )
                # outT[c, n] = sum_pairs wvo_pair^T @ embT_pair
                for g in range(8):
                    pout = psp.tile([128, 512], dt.float32, tag="pp",
                                    name=f"pout{inst}_{g}")
                    for k in range(4):
                        nc.tensor.matmul(
                            pout[0:C, :],
                            wvo2[:, k, :],
                            etp_tiles[k][:, g * 512:(g + 1) * 512],
                            start=(k == 0),
                            stop=(k == 3),
                        )
                    ot = streamp.tile([C, 512], dt.float32, tag="ot",
                                      bufs=2)
                    # DVE-only drains: any ACT op here would clock-gate
                    # the PE (K=4) during the pure-matmul endgame.
                    nc.vector.tensor_copy(out=ot[:], in_=pout[0:C, :])
                    nc.sync.dma_start(
                        out_d[inst, :, g * 512:(g + 1) * 512], ot[:]
                    )

            # ================= schedule =================
            # m phases chase the input stream: dense PE chains per pair
            # (both instances); ssq partials overlap the load window.
            for pair in range(4):
                e2ts = [m_phase_pe(inst, pair) for inst in range(2)]
                for inst in range(2):
                    m_phase_m(inst, pair, e2ts[inst])
                for inst in range(2):
                    pair_ssq(inst, pair)
            stats_prep_both()
            stage_etp()

            # middle: ACT-bound exp-drain stream; the dep-driven scheduler
            # pulls the wva matmuls into the PE gaps as planes drain.
            for db in range(B):
                sT_fill(0, db)
            for db in range(B):
                sT_fill(1, db)
            for db in range(B):
                wva(0, db)
                wva(1, db)

            ctx_out(0)
            ctx_out(1)

    nc.compile()
    return nc


def _get_nc():
    global _nc
    if _nc is None:
        _nc = _build()
    return _nc


def kernel(emb, Wq, Wk, Wv, Wout):
    emb = np.ascontiguousarray(emb, dtype=np.float32)
    Wq = np.ascontiguousarray(Wq, dtype=np.float32)
    Wk = np.ascontiguousarray(Wk, dtype=np.float32)
    Wv = np.ascontiguousarray(Wv, dtype=np.float32)
    Wout = np.ascontiguousarray(Wout, dtype=np.float32)
    emb_l, emb_u = emb[:B], emb[B:]
    ident = np.eye(128, dtype=np.float32)
    ones = np.ones((128, 128), dtype=np.float32)

    in_maps = []
    for core in range(8):
        if core < 4:
            qb, kvb = emb_l[2 * core:2 * core + 2], emb_u
        else:
            j = core - 4
            qb, kvb = emb_u[2 * j:2 * j + 2], emb_l
        in_maps.append({
            "embq": np.ascontiguousarray(qb), "embkv": np.ascontiguousarray(kvb),
            "Wq": Wq, "Wk": Wk, "Wv": Wv, "Wout": Wout, "ident": ident,
            "ones": ones,
        })

    res = run_bass_kernel_spmd(_get_nc(), in_maps, list(range(8))).results

    out = np.empty((2 * B, N, C), np.float32)
    for core in range(8):
        o = res[core]["out"].transpose(0, 2, 1)  # [2, C, N] -> [2, N, C]
        if core < 4:
            out[2 * core:2 * core + 2] = o
        else:
            j = core - 4
            out[B + 2 * j:B + 2 * j + 2] = o
    return out


# revision 32
# speedup vs baseline: 1.0752x; 1.0617x over previous
"""Cross-attention (global, batch-flattened K/V) Trainium2 kernel, v7.

Problem: emb [16, 4096, 64]; two cross-attention halves:
  out_l2u = cross(q=emb[:8],  kv=emb[8:])   -> rows 0..7
  out_u2l = cross(q=emb[8:],  kv=emb[:8])   -> rows 8..15
cross(): q/k/v proj (64->512), s = einsum('bnc,nd->bcd', q, kflat),
InstanceNorm over (CH, B*CH) plane per b, softmax over d, ctx = a @ vflat^T,
out = ctx @ Wout.

Sharding: 16 independent (cross, q-batch) instances, 2 per core.
Cores 0-3: q from lower half (kv = upper), cores 4-7: q from upper
(kv = lower). No collectives; weights replicated.

Rank-64 factoring (scores have rank <= 64):
  sT[d,:] for kv batch db = Wk^T @ (emb_db^T @ q)      (m = emb^T q Wq)
  outT = sum_db (wvo_db)^T @ emb_db^T,  wvo_db = (Wv @ aT_db / den) @ Wout
InstanceNorm stats come analytically from m (plane sum via Wk row sums,
sum-of-squares via <m, G2m @ m> with G2m = blockdiag(Wk Wk^T)), so they
are available BEFORE the score plane exists.  Schedule:
  1. m phases chase the streaming input DMA pair by pair, both
     instances back-to-back so the PE runs dense chains (no ping-pong).
  2. Middle is ACT-bound only: score blocks are drained PSUM->SBUF with
     exp(scale*x+bias) FUSED into the drain; inst0's wva matmuls fill
     the PE while inst1's plane drains.
  3. Endgame (wva(1), pwo, outT) is a pure dense PE stretch.
Output stores go out on the gpsimd DMA queue, XBAR transposes early on
sync, so no queue blocks another.
"""

import numpy as np
import concourse.bass as bass
import concourse.mybir as mybir
import concourse.tile as tile
from concourse import bacc
from concourse.bass_utils import run_bass_kernel_spmd

dt = mybir.dt
AF = mybir.ActivationFunctionType
ALU = mybir.AluOpType

B = 8            # batches per half
N = 4096         # sequence length
C = 64           # embedding channels
CH = 512         # num_heads * C
NB = N // 128    # 32 n-blocks
CB = CH // 128   # 4 c-blocks
D = B * CH       # 4096 flattened kv dim
EPS = 1e-5
BF = dt.bfloat16
PLANE = float(CH * D)  # InstanceNorm plane size per instance

_nc = None


def _build():
    nc = bacc.Bacc("TRN2", target_bir_lowering=False, debug=False, num_devices=8)

    embq = nc.declare_dram_parameter("embq", [2, N, C], dt.float32, isOutput=False)
    embkv = nc.declare_dram_parameter("embkv", [B, N, C], dt.float32, isOutput=False)
    Wq_d = nc.declare_dram_parameter("Wq", [C, CH], dt.float32, isOutput=False)
    Wk_d = nc.declare_dram_parameter("Wk", [C, CH], dt.float32, isOutput=False)
    Wv_d = nc.declare_dram_parameter("Wv", [C, CH], dt.float32, isOutput=False)
    Wout_d = nc.declare_dram_parameter("Wout", [CH, C], dt.float32, isOutput=False)
    ident_d = nc.declare_dram_parameter("ident", [128, 128], dt.float32, isOutput=False)
    ones_d = nc.declare_dram_parameter("ones", [128, 128], dt.float32, isOutput=False)
    out_d = nc.declare_dram_parameter("out", [2, C, N], dt.float32, isOutput=True)

    # bf16 kv emb, pair-packed: slot k = kv batches (2k | 2k+1) in cols
    # (0:64 | 64:128); staged in DRAM only for the XBAR-transposed reads.
    # Separate tensors per pair keep the write->transpose deps independent.
    emb_bfs = [nc.dram_tensor(f"emb_bf{k}", [N, 128], BF) for k in range(4)]

    with tile.TileContext(nc) as tc:
        with (
            tc.tile_pool(name="const", bufs=1) as constp,
            tc.tile_pool(name="io", bufs=2) as iop,
            tc.tile_pool(name="res", bufs=1) as resp,
            tc.tile_pool(name="stream", bufs=2) as streamp,
            tc.tile_pool(name="small", bufs=1) as smallp,
            tc.tile_pool(name="ps", bufs=2, space="PSUM") as psp,
        ):
            # ===== input DMA issue (sync queue): q0, kv pair 0 first so
            # the m phase starts as early as possible, then weights, then
            # the rest of the stream. =====
            lt_q0 = iop.tile([128, NB, C], dt.float32, tag="ld", bufs=3)
            nc.sync.dma_start(
                lt_q0[:], embq[0].rearrange("(p nb) c -> p nb c", p=128)
            )
            lt_kv0 = iop.tile([128, NB, C], dt.float32, tag="ld", bufs=3)
            nc.scalar.dma_start(
                lt_kv0[:], embkv[0].rearrange("(p nb) c -> p nb c", p=128)
            )
            lt_kv1 = iop.tile([128, NB, C], dt.float32, tag="ld", bufs=3)
            nc.sync.dma_start(
                lt_kv1[:], embkv[1].rearrange("(p nb) c -> p nb c", p=128)
            )

            ident = constp.tile([128, 128], dt.float32, tag="ident")
            nc.sync.dma_start(ident[:], ident_d[:])
            ones_f = iop.tile([128, 128], dt.float32, tag="wst", bufs=4)
            nc.sync.dma_start(ones_f[:], ones_d[:])
            w_st = {}
            for name, wd in (("Wq", Wq_d), ("Wv", Wv_d)):
                wst = iop.tile([C, CH], dt.float32, tag="wst", bufs=4)
                nc.sync.dma_start(wst[:], wd[:])
                w_st[name] = wst
            wk2st = constp.tile([128, CH], dt.float32, tag="wk2st")
            nc.sync.dma_start(wk2st[0:C, :], Wk_d[:])
            nc.sync.dma_start(wk2st[C:128, :], Wk_d[:])
            wost = iop.tile([128, CB, C], dt.float32, tag="wst", bufs=4)
            nc.sync.dma_start(
                wost[:], Wout_d[:].rearrange("(cb p) c -> p cb c", p=128)
            )

            lt_q1 = iop.tile([128, NB, C], dt.float32, tag="ld", bufs=3)
            nc.scalar.dma_start(
                lt_q1[:], embq[1].rearrange("(p nb) c -> p nb c", p=128)
            )
            ltq = [lt_q0, lt_q1]
            ltkv = [lt_kv0, lt_kv1]
            for b in range(2, B):
                lt = iop.tile([128, NB, C], dt.float32, tag="ld", bufs=3)
                eng = nc.sync if b % 2 == 0 else nc.scalar
                eng.dma_start(
                    lt[:], embkv[b].rearrange("(p nb) c -> p nb c", p=128)
                )
                ltkv.append(lt)

            # ===== constants / weight prep =====
            ones_r = constp.tile([128, 128], dt.float32r, tag="ones_r")
            nc.vector.tensor_copy(out=ones_r[:], in_=ones_f[:])
            onescol = constp.tile([128, 1], BF, tag="onescol")
            nc.vector.tensor_copy(out=onescol[:], in_=ones_f[:, 0:1])

            Wq_b = constp.tile([C, CH], BF, tag="Wq_bf")
            nc.vector.tensor_copy(out=Wq_b[:], in_=w_st["Wq"][:])
            # Wk duplicated on both partition halves so the sT matmul can
            # read m for the odd batch of a pair at base partition 64
            Wk2_b = constp.tile([128, CH], BF, tag="Wk2_bf")
            nc.vector.tensor_copy(out=Wk2_b[:], in_=wk2st[:])
            # row sums of Wk2 (for plane-sum of s via m)
            wk2s = constp.tile([128, 1], dt.float32, tag="wk2s")
            nc.vector.tensor_reduce(
                out=wk2s[:], in_=Wk2_b[:], axis=mybir.AxisListType.X,
                op=ALU.add,
            )
            # G2m = blockdiag(Wk Wk^T, Wk Wk^T)  (for plane sum-of-squares)
            wk2t_sb = constp.tile([128, CB, 128], BF, tag="wk2t")
            for k in range(CB):
                pbt = psp.tile([128, 512], dt.float32, tag="pp", bufs=2)
                nc.tensor.transpose(
                    pbt[:, 0:128],
                    wk2st[:, k * 128:(k + 1) * 128],
                    ident[:],
                )
                nc.vector.tensor_copy(out=wk2t_sb[:, k, :], in_=pbt[:, 0:128])
            ptg = psp.tile([128, 512], dt.float32, tag="pp", name="ptg")
            for k in range(CB):
                nc.tensor.matmul(
                    ptg[:, 0:128], wk2t_sb[:, k, :], wk2t_sb[:, k, :],
                    start=(k == 0), stop=(k == CB - 1),
                )
            G2m_b = constp.tile([128, 128], BF, tag="G2m_b")
            nc.vector.tensor_copy(out=G2m_b[:], in_=ptg[:, 0:128])
            nc.vector.memset(G2m_b[0:C, C:128], 0.0)
            nc.vector.memset(G2m_b[C:128, 0:C], 0.0)

            # WvT [128(ch sub), CB, 65] via 4 fp32 PE transposes; the 65th
            # column is ones so the wva matmuls also produce the softmax
            # denominator partials.
            WvT_b = constp.tile([128, CB, C + 1], BF, tag="WvT_bf")
            ptw = psp.tile([128, 512], dt.float32, tag="pp")
            for k in range(CB):
                nc.tensor.transpose(
                    ptw[:, k * 128:k * 128 + C],
                    w_st["Wv"][:, k * 128:(k + 1) * 128],
                    ident[0:C, 0:C],
                )
            for k in range(CB):
                nc.vector.tensor_copy(
                    out=WvT_b[:, k, 0:C], in_=ptw[:, k * 128:k * 128 + C]
                )
                nc.vector.tensor_copy(
                    out=WvT_b[:, k, C:C + 1], in_=onescol[:]
                )

            Wout_b = constp.tile([128, CB, C], BF, tag="Wout_bf")
            nc.vector.tensor_copy(out=Wout_b[:], in_=wost[:])

            # ===== streamed casts + staging =====
            ebq_tiles = []
            for inst in range(2):
                ebq = iop.tile([128, NB, C], BF, tag="ebq", bufs=2)
                nc.vector.tensor_copy(out=ebq[:], in_=ltq[inst][:])
                ebq_tiles.append(ebq)

            lb_tiles = []
            for pair in range(4):
                lb = iop.tile([128, NB, 128], BF, tag="ldb", bufs=4)
                # both cast halves on ACT: the DVE is the m-phase pacer,
                # and a DVE backlog here stalls the input-load ring.
                nc.scalar.activation(lb[:, :, 0:C], ltkv[2 * pair][:],
                                     AF.Copy)
                nc.scalar.activation(lb[:, :, C:128], ltkv[2 * pair + 1][:],
                                     AF.Copy)
                lb_tiles.append(lb)

            # Staging gate: a 1-element DMA into each emb_bf tensor whose
            # source depends on the LAST pair's cast.  The 1MB staging
            # writes (WAW on that element) and the XBAR transposes are
            # thereby deferred until the input loads have drained, so they
            # don't steal HBM bandwidth from the critical input stream.
            # Everything lives on the sync ring (emitted later, after the
            # stats) so the ACT ring never stalls behind a staging issue.
            gate_sb = smallp.tile([1, 4], BF, tag="gate")
            nc.vector.tensor_tensor(
                out=gate_sb[:], in0=lb_tiles[3][0:1, 0, 0:4],
                in1=lb_tiles[3][0:1, 0, 0:4], op=ALU.bypass,
            )
            etp_tiles = []

            def stage_etp():
                for pair in range(4):
                    nc.sync.dma_start(
                        emb_bfs[pair][0:1, 0:1], gate_sb[0:1, pair:pair + 1]
                    )
                for pair in range(4):
                    nc.sync.dma_start(
                        emb_bfs[pair].rearrange("(p nb) c -> p nb c", p=128),
                        lb_tiles[pair][:],
                    )
                for pair in range(4):
                    etp = streamp.tile([128, N], BF, tag="etp", bufs=4)
                    nc.sync.dma_start_transpose(etp[:], emb_bfs[pair][:])
                    etp_tiles.append(etp)

            # ===== persistent small tensors =====
            ssum = smallp.tile([128, 2, 4], dt.float32, tag="ssum")
            ssq = smallp.tile([128, 2, 4], dt.float32, tag="ssq")
            gmtmp = smallp.tile([128, CH], dt.float32, tag="gmtmp")
            mrs = smallp.tile([128, 1], dt.float32, tag="mrs", bufs=2)
            stats2 = smallp.tile([128, 8, 2], dt.float32, tag="stats2")
            invden2 = smallp.tile([128, 2, CB], dt.float32, tag="invden2")
            den_acc = [
                smallp.tile([128, CB, 1], dt.float32, tag="denacc", bufs=2,
                            name=f"den{i}")
                for i in range(2)
            ]

            m_tiles = {}

            def m_phase_pe(inst, pair):
                """E2T = emb_q^T @ [emb_2p | emb_2p+1]  ([64, 128])."""
                pE = psp.tile([128, 512], dt.float32, tag="pp",
                              name=f"pE{inst}_{pair}")
                lb = lb_tiles[pair]
                ebq = ebq_tiles[inst]
                for nb in range(NB):
                    nc.tensor.matmul(
                        pE[0:C, 0:128],
                        ebq[:, nb, :],
                        lb[:, nb, :],
                        start=(nb == 0),
                        stop=(nb == NB - 1),
                    )
                e2t = streamp.tile([C, 128], BF, tag="e2t")
                nc.vector.tensor_copy(out=e2t[:], in_=pE[0:C, 0:128])
                return e2t

            def m_phase_m(inst, pair, e2t):
                """m = E2T^T @ Wq ([128, CH]); plane-sum partials."""
                pm = psp.tile([128, 512], dt.float32, tag="pp",
                              name=f"pm{inst}_{pair}")
                nc.tensor.matmul(
                    pm[:], e2t[:], Wq_b[:], start=True, stop=True
                )
                m_sb = streamp.tile([128, CH], BF, tag="msb", bufs=8)
                nc.vector.tensor_copy(out=m_sb[:], in_=pm[:])
                m_tiles[(inst, pair)] = m_sb
                nc.vector.tensor_reduce(
                    out=mrs[:], in_=m_sb[:], axis=mybir.AxisListType.X,
                    op=ALU.add,
                )
                nc.vector.tensor_tensor(
                    out=ssum[:, inst, pair:pair + 1], in0=mrs[:],
                    in1=wk2s[:], op=ALU.mult,
                )

            def pair_ssq(inst, pair):
                """plane sum-of-squares partial: <m, G2m @ m>."""
                m_sb = m_tiles[(inst, pair)]
                pgm = psp.tile([128, 512], dt.float32, tag="pp",
                               name=f"pgm{inst}_{pair}")
                nc.tensor.matmul(
                    pgm[:], G2m_b[:], m_sb[:], start=True, stop=True
                )
                nc.vector.tensor_tensor(
                    out=gmtmp[:], in0=m_sb[:], in1=pgm[:], op=ALU.mult,
                )
                nc.vector.tensor_reduce(
                    out=ssq[:, inst, pair:pair + 1], in_=gmtmp[:],
                    axis=mybir.AxisListType.X, op=ALU.add,
                )

            def stats_prep_both():
                """InstanceNorm stats for BOTH instances in one pass.
                Everything on DVE except a single [128, 2] Sqrt, so the
                ACT engine sees at most one table switch before the exp
                stream (table thrash mid-stream costs 1.3us per switch).
                stats2 layout: [128, stat, inst]."""
                red = smallp.tile([128, 4], dt.float32, tag="red")
                for inst in range(2):
                    nc.vector.tensor_reduce(
                        out=red[:, 2 * inst:2 * inst + 1],
                        in_=ssum[:, inst, :],
                        axis=mybir.AxisListType.X, op=ALU.add,
                    )
                    nc.vector.tensor_reduce(
                        out=red[:, 2 * inst + 1:2 * inst + 2],
                        in_=ssq[:, inst, :],
                        axis=mybir.AxisListType.X, op=ALU.add,
                    )
                red_r = smallp.tile([128, 4], dt.float32r, tag="red_r")
                nc.vector.tensor_copy(out=red_r[:], in_=red[:])
                ptr = psp.tile([128, 512], dt.float32, tag="pp", name="ptr")
                nc.tensor.matmul(
                    ptr[:, 0:4], ones_r[:], red_r[:], start=True, stop=True
                )
                # ptr cols = (sum0, sumsq0, sum1, sumsq1) -> stat-major
                nc.vector.tensor_scalar_mul(
                    stats2[:, 0:2, :],
                    ptr[:, 0:4].rearrange("p (i k) -> p k i", i=2),
                    1.0 / PLANE,
                )
                mu = stats2[:, 0, :]
                ex2 = stats2[:, 1, :]
                musq = stats2[:, 2, :]
                var = stats2[:, 3, :]
                std = stats2[:, 4, :]
                rstd = stats2[:, 5, :]
                nmr = stats2[:, 6, :]
                nc.vector.tensor_tensor(out=musq, in0=mu, in1=mu, op=ALU.mult)
                nc.vector.tensor_tensor(out=var, in0=ex2, in1=musq,
                                        op=ALU.subtract)
                nc.vector.tensor_scalar_add(var, var, EPS)
                nc.scalar.activation(std, var, AF.Sqrt, bias=0.0)
                nc.vector.reciprocal(rstd, std)
                nc.vector.tensor_tensor(out=nmr, in0=mu, in1=rstd,
                                        op=ALU.mult)
                nc.vector.tensor_scalar_mul(nmr, nmr, -1.0)

            sT_tiles = [
                resp.tile([128, NB, CH], BF, tag="sT", bufs=2,
                          name=f"sT{i}")
                for i in range(2)
            ]

            def sT_fill(inst, db):
                """sT[d, c] = exp(rstd * Wk^T m - mu*rstd): two 2-bank PSUM
                chunks, each drained [128, 1024] by ACT with fused exp."""
                half = (db % 2) * C
                m_sb = m_tiles[(inst, db // 2)]
                sT = sT_tiles[inst]
                for c in range(2):
                    ps2 = psp.tile([128, 2, 512], dt.float32, tag="ps2",
                                   bufs=3, name=f"ps2_{inst}_{db}_{c}")
                    for d2 in range(2):
                        dc = 2 * c + d2
                        nc.tensor.matmul(
                            ps2[:, d2, :],
                            Wk2_b[half:half + C, dc * 128:(dc + 1) * 128],
                            m_sb[half:half + C, :],
                            start=True,
                            stop=True,
                        )
                    kb = db * CB + 2 * c
                    nc.scalar.activation(
                        sT[:, kb:kb + 2, :], ps2[:], AF.Exp,
                        bias=stats2[:, 6, inst:inst + 1],
                        scale=stats2[:, 5, inst:inst + 1],
                    )

            wva_tiles = [
                streamp.tile([128, B, CB, C], BF, tag="wva8", bufs=2,
                             name=f"wva{i}")
                for i in range(2)
            ]

            def wva(inst, db):
                """wva_db = (a_db Wv^T | den partials): 16 PE matmuls off
                the exp'd sT block, drained by a single DVE copy."""
                sT = sT_tiles[inst]
                W65 = C + 1
                pwa = psp.tile([128, 512], dt.float32, tag="pp",
                               name=f"pwa{inst}_{db}")
                for chb in range(CB):
                    for j in range(CB):
                        nc.tensor.matmul(
                            pwa[:, chb * 128:chb * 128 + W65],
                            sT[:, db * CB + j, chb * 128:(chb + 1) * 128],
                            WvT_b[:, j, :],
                            start=(j == 0),
                            stop=(j == CB - 1),
                        )
                wva_all = wva_tiles[inst]
                pview = pwa[:].rearrange("p (a b) -> p a b", b=128)
                nc.vector.tensor_copy(
                    out=wva_all[:, db, :, :], in_=pview[:, :, 0:C],
                )
                dcol = pview[:, :, C:C + 1]
                if db == 0:
                    nc.vector.tensor_copy(out=den_acc[inst][:], in_=dcol)
                else:
                    nc.vector.tensor_tensor(
                        out=den_acc[inst][:], in0=den_acc[inst][:],
                        in1=dcol, op=ALU.add,
                    )

            def ctx_out(inst):
                """outT = sum_db (wvo_db)^T @ emb_db^T with
                wvo_db = wva_db @ (Wout / den)."""
                wva_all = wva_tiles[inst]
                nc.vector.reciprocal(
                    invden2[:, inst, :], den_acc[inst][:, :, 0]
                )
                wout_s = streamp.tile([128, CB, C], BF, tag="wout_s",
                                      bufs=2)
                for k in range(CB):
                    nc.vector.tensor_scalar_mul(
                        wout_s[:, k, :], Wout_b[:, k, :],
                        invden2[:, inst, k:k + 1],
                    )
                pwo = psp.tile([128, 512], dt.float32, tag="pp",
                               name=f"pwo{inst}")
                for db in range(B):
                    for chb in range(CB):
                        nc.tensor.matmul(
                            pwo[0:C, db * C:(db + 1) * C],
                            wva_all[:, db, chb, :],
                            wout_s[:, chb, :],
                            start=(chb == 0),
                            stop=(chb == CB - 1),
                        )
                # pair-stack wvo: [128(2 batches' c'), 4(pair), 64]
                wvo2 = streamp.tile([128, CB, C], BF, tag="wvo2", bufs=2)
                for db in range(B):
                    nc.vector.tensor_copy(
                        out=wvo2[(db % 2) * C:(db % 2 + 1) * C, db // 2, :],
                        in_=pwo[0:C, db * C:(db + 1) * C],
                    )
                # outT[c, n] = sum_pairs wvo_pair^T @ embT_pair
                for g in range(8):
                    pout = psp.tile([128, 512], dt.float32, tag="pp",
                                    name=f"pout{inst}_{g}")
                    for k in range(4):
                        nc.tensor.matmul(
                            pout[0:C, :],
                            wvo2[:, k, :],
                            etp_tiles[k][:, g * 512:(g + 1) * 512],
                            start=(k == 0),
                            stop=(k == 3),
                        )
                    ot = streamp.tile([C, 512], dt.float32, tag="ot",
                                      bufs=2)
                    # DVE-only drains: any ACT op here would clock-gate
                    # the PE (K=4) during the pure-matmul endgame.
                    nc.vector.tensor_copy(out=ot[:], in_=pout[0:C, :])
                    nc.sync.dma_start(
                        out_d[inst, :, g * 512:(g + 1) * 512], ot[:]
                    )

            # ================= schedule =================
            # m phases chase the input stream: dense PE chains per pair
            # (both instances); ssq partials overlap the load window.
            for pair in range(4):
                e2ts = [m_phase_pe(inst, pair) for inst in range(2)]
                for inst in range(2):
                    m_phase_m(inst, pair, e2ts[inst])
                for inst in range(2):
                    pair_ssq(inst, pair)
            stats_prep_both()
            stage_etp()

            # middle: ACT-bound exp-drain stream, fills only.
            for db in range(B):
                sT_fill(0, db)
            for db in range(B):
                sT_fill(1, db)
            # Gate the wva matmuls behind the LAST exp drain (bypass write
            # into a corner each wva reads): during the exp stream the PE
            # is clock-gated to 1.2GHz, so running wva there costs double;
            # deferred, it runs in the ACT-quiet endgame at 2.4GHz.
            for i in range(2):
                nc.vector.tensor_tensor(
                    out=sT_tiles[i][0:1, 0, 0:1],
                    in0=sT_tiles[i][0:1, 0, 0:1],
                    in1=sT_tiles[1][0:1, NB - 1, CH - 1:CH],
                    op=ALU.bypass,
                )
            for db in range(B):
                wva(0, db)
                wva(1, db)

            ctx_out(0)
            ctx_out(1)

    nc.compile()
    return nc


def _get_nc():
    global _nc
    if _nc is None:
        _nc = _build()
    return _nc


def kernel(emb, Wq, Wk, Wv, Wout):
    emb = np.ascontiguousarray(emb, dtype=np.float32)
    Wq = np.ascontiguousarray(Wq, dtype=np.float32)
    Wk = np.ascontiguousarray(Wk, dtype=np.float32)
    Wv = np.ascontiguousarray(Wv, dtype=np.float32)
    Wout = np.ascontiguousarray(Wout, dtype=np.float32)
    emb_l, emb_u = emb[:B], emb[B:]
    ident = np.eye(128, dtype=np.float32)
    ones = np.ones((128, 128), dtype=np.float32)

    in_maps = []
    for core in range(8):
        if core < 4:
            qb, kvb = emb_l[2 * core:2 * core + 2], emb_u
        else:
            j = core - 4
            qb, kvb = emb_u[2 * j:2 * j + 2], emb_l
        in_maps.append({
            "embq": np.ascontiguousarray(qb), "embkv": np.ascontiguousarray(kvb),
            "Wq": Wq, "Wk": Wk, "Wv": Wv, "Wout": Wout, "ident": ident,
            "ones": ones,
        })

    res = run_bass_kernel_spmd(_get_nc(), in_maps, list(range(8))).results

    out = np.empty((2 * B, N, C), np.float32)
    for core in range(8):
        o = res[core]["out"].transpose(0, 2, 1)  # [2, C, N] -> [2, N, C]
        if core < 4:
            out[2 * core:2 * core + 2] = o
        else:
            j = core - 4
            out[B + 2 * j:B + 2 * j + 2] = o
    return out


# revision 33
# speedup vs baseline: 1.0908x; 1.0146x over previous
"""Cross-attention (global, batch-flattened K/V) Trainium2 kernel, v7.

Problem: emb [16, 4096, 64]; two cross-attention halves:
  out_l2u = cross(q=emb[:8],  kv=emb[8:])   -> rows 0..7
  out_u2l = cross(q=emb[8:],  kv=emb[:8])   -> rows 8..15
cross(): q/k/v proj (64->512), s = einsum('bnc,nd->bcd', q, kflat),
InstanceNorm over (CH, B*CH) plane per b, softmax over d, ctx = a @ vflat^T,
out = ctx @ Wout.

Sharding: 16 independent (cross, q-batch) instances, 2 per core.
Cores 0-3: q from lower half (kv = upper), cores 4-7: q from upper
(kv = lower). No collectives; weights replicated.

Rank-64 factoring (scores have rank <= 64):
  sT[d,:] for kv batch db = Wk^T @ (emb_db^T @ q)      (m = emb^T q Wq)
  outT = sum_db (wvo_db)^T @ emb_db^T,  wvo_db = (Wv @ aT_db / den) @ Wout
InstanceNorm stats come analytically from m (plane sum via Wk row sums,
sum-of-squares via <m, G2m @ m> with G2m = blockdiag(Wk Wk^T)), so they
are available BEFORE the score plane exists.  Schedule:
  1. m phases chase the streaming input DMA pair by pair, both
     instances back-to-back so the PE runs dense chains (no ping-pong).
  2. Middle is ACT-bound only: score blocks are drained PSUM->SBUF with
     exp(scale*x+bias) FUSED into the drain; inst0's wva matmuls fill
     the PE while inst1's plane drains.
  3. Endgame (wva(1), pwo, outT) is a pure dense PE stretch.
Output stores go out on the gpsimd DMA queue, XBAR transposes early on
sync, so no queue blocks another.
"""

import numpy as np
import concourse.bass as bass
import concourse.mybir as mybir
import concourse.tile as tile
from concourse import bacc
from concourse.bass_utils import run_bass_kernel_spmd

dt = mybir.dt
AF = mybir.ActivationFunctionType
ALU = mybir.AluOpType

B = 8            # batches per half
N = 4096         # sequence length
C = 64           # embedding channels
CH = 512         # num_heads * C
NB = N // 128    # 32 n-blocks
CB = CH // 128   # 4 c-blocks
D = B * CH       # 4096 flattened kv dim
EPS = 1e-5
BF = dt.bfloat16
PLANE = float(CH * D)  # InstanceNorm plane size per instance

_nc = None


def _build():
    nc = bacc.Bacc("TRN2", target_bir_lowering=False, debug=False, num_devices=8)

    embq = nc.declare_dram_parameter("embq", [2, N, C], dt.float32, isOutput=False)
    embkv = nc.declare_dram_parameter("embkv", [B, N, C], dt.float32, isOutput=False)
    Wq_d = nc.declare_dram_parameter("Wq", [C, CH], dt.float32, isOutput=False)
    Wk_d = nc.declare_dram_parameter("Wk", [C, CH], dt.float32, isOutput=False)
    Wv_d = nc.declare_dram_parameter("Wv", [C, CH], dt.float32, isOutput=False)
    Wout_d = nc.declare_dram_parameter("Wout", [CH, C], dt.float32, isOutput=False)
    ident_d = nc.declare_dram_parameter("ident", [128, 128], dt.float32, isOutput=False)
    ones_d = nc.declare_dram_parameter("ones", [128, 128], dt.float32, isOutput=False)
    out_d = nc.declare_dram_parameter("out", [2, C, N], dt.float32, isOutput=True)

    # bf16 kv emb, pair-packed: slot k = kv batches (2k | 2k+1) in cols
    # (0:64 | 64:128); staged in DRAM only for the XBAR-transposed reads.
    # Separate tensors per pair keep the write->transpose deps independent.
    emb_bfs = [nc.dram_tensor(f"emb_bf{k}", [N, 128], BF) for k in range(4)]

    with tile.TileContext(nc) as tc:
        with (
            tc.tile_pool(name="const", bufs=1) as constp,
            tc.tile_pool(name="io", bufs=2) as iop,
            tc.tile_pool(name="res", bufs=1) as resp,
            tc.tile_pool(name="stream", bufs=2) as streamp,
            tc.tile_pool(name="small", bufs=1) as smallp,
            tc.tile_pool(name="ps", bufs=2, space="PSUM") as psp,
        ):
            # ===== input DMA issue (sync queue): q0, kv pair 0 first so
            # the m phase starts as early as possible, then weights, then
            # the rest of the stream. =====
            lt_q0 = iop.tile([128, NB, C], dt.float32, tag="ld", bufs=3)
            nc.sync.dma_start(
                lt_q0[:], embq[0].rearrange("(p nb) c -> p nb c", p=128)
            )
            lt_kv0 = iop.tile([128, NB, C], dt.float32, tag="ld", bufs=3)
            nc.scalar.dma_start(
                lt_kv0[:], embkv[0].rearrange("(p nb) c -> p nb c", p=128)
            )
            lt_kv1 = iop.tile([128, NB, C], dt.float32, tag="ld", bufs=3)
            nc.sync.dma_start(
                lt_kv1[:], embkv[1].rearrange("(p nb) c -> p nb c", p=128)
            )

            ident = constp.tile([128, 128], dt.float32, tag="ident")
            nc.sync.dma_start(ident[:], ident_d[:])
            ones_f = iop.tile([128, 128], dt.float32, tag="wst", bufs=4)
            nc.sync.dma_start(ones_f[:], ones_d[:])
            w_st = {}
            for name, wd in (("Wq", Wq_d), ("Wv", Wv_d)):
                wst = iop.tile([C, CH], dt.float32, tag="wst", bufs=4)
                nc.sync.dma_start(wst[:], wd[:])
                w_st[name] = wst
            wk2st = constp.tile([128, CH], dt.float32, tag="wk2st")
            nc.sync.dma_start(wk2st[0:C, :], Wk_d[:])
            nc.sync.dma_start(wk2st[C:128, :], Wk_d[:])
            wost = iop.tile([128, CB, C], dt.float32, tag="wst", bufs=4)
            nc.sync.dma_start(
                wost[:], Wout_d[:].rearrange("(cb p) c -> p cb c", p=128)
            )

            lt_q1 = iop.tile([128, NB, C], dt.float32, tag="ld", bufs=3)
            nc.scalar.dma_start(
                lt_q1[:], embq[1].rearrange("(p nb) c -> p nb c", p=128)
            )
            ltq = [lt_q0, lt_q1]
            ltkv = [lt_kv0, lt_kv1]
            for b in range(2, B):
                lt = iop.tile([128, NB, C], dt.float32, tag="ld", bufs=3)
                eng = nc.sync if b % 2 == 0 else nc.scalar
                eng.dma_start(
                    lt[:], embkv[b].rearrange("(p nb) c -> p nb c", p=128)
                )
                ltkv.append(lt)

            # ===== constants / weight prep =====
            ones_r = constp.tile([128, 128], dt.float32r, tag="ones_r")
            nc.vector.tensor_copy(out=ones_r[:], in_=ones_f[:])
            onescol = constp.tile([128, 1], BF, tag="onescol")
            nc.vector.tensor_copy(out=onescol[:], in_=ones_f[:, 0:1])

            Wq_b = constp.tile([C, CH], BF, tag="Wq_bf")
            nc.vector.tensor_copy(out=Wq_b[:], in_=w_st["Wq"][:])
            # Wk duplicated on both partition halves so the sT matmul can
            # read m for the odd batch of a pair at base partition 64
            Wk2_b = constp.tile([128, CH], BF, tag="Wk2_bf")
            nc.vector.tensor_copy(out=Wk2_b[:], in_=wk2st[:])
            # row sums of Wk2 (for plane-sum of s via m)
            wk2s = constp.tile([128, 1], dt.float32, tag="wk2s")
            nc.vector.tensor_reduce(
                out=wk2s[:], in_=Wk2_b[:], axis=mybir.AxisListType.X,
                op=ALU.add,
            )
            # G2m = blockdiag(Wk Wk^T, Wk Wk^T)  (for plane sum-of-squares)
            wk2t_sb = constp.tile([128, CB, 128], BF, tag="wk2t")
            for k in range(CB):
                pbt = psp.tile([128, 512], dt.float32, tag="pp", bufs=2)
                nc.tensor.transpose(
                    pbt[:, 0:128],
                    wk2st[:, k * 128:(k + 1) * 128],
                    ident[:],
                )
                nc.vector.tensor_copy(out=wk2t_sb[:, k, :], in_=pbt[:, 0:128])
            ptg = psp.tile([128, 512], dt.float32, tag="pp", name="ptg")
            for k in range(CB):
                nc.tensor.matmul(
                    ptg[:, 0:128], wk2t_sb[:, k, :], wk2t_sb[:, k, :],
                    start=(k == 0), stop=(k == CB - 1),
                )
            G2m_b = constp.tile([128, 128], BF, tag="G2m_b")
            nc.vector.tensor_copy(out=G2m_b[:], in_=ptg[:, 0:128])
            nc.vector.memset(G2m_b[0:C, C:128], 0.0)
            nc.vector.memset(G2m_b[C:128, 0:C], 0.0)

            # WvT [128(ch sub), CB, 65] via 4 fp32 PE transposes; the 65th
            # column is ones so the wva matmuls also produce the softmax
            # denominator partials.
            WvT_b = constp.tile([128, CB, C + 1], BF, tag="WvT_bf")
            ptw = psp.tile([128, 512], dt.float32, tag="pp")
            for k in range(CB):
                nc.tensor.transpose(
                    ptw[:, k * 128:k * 128 + C],
                    w_st["Wv"][:, k * 128:(k + 1) * 128],
                    ident[0:C, 0:C],
                )
            for k in range(CB):
                nc.vector.tensor_copy(
                    out=WvT_b[:, k, 0:C], in_=ptw[:, k * 128:k * 128 + C]
                )
                nc.vector.tensor_copy(
                    out=WvT_b[:, k, C:C + 1], in_=onescol[:]
                )

            Wout_b = constp.tile([128, CB, C], BF, tag="Wout_bf")
            nc.vector.tensor_copy(out=Wout_b[:], in_=wost[:])

            # ===== streamed casts + staging =====
            ebq_tiles = []
            for inst in range(2):
                ebq = iop.tile([128, NB, C], BF, tag="ebq", bufs=2)
                nc.vector.tensor_copy(out=ebq[:], in_=ltq[inst][:])
                ebq_tiles.append(ebq)

            lb_tiles = []
            for pair in range(4):
                lb = iop.tile([128, NB, 128], BF, tag="ldb", bufs=4)
                # both cast halves on ACT: the DVE is the m-phase pacer,
                # and a DVE backlog here stalls the input-load ring.
                nc.scalar.activation(lb[:, :, 0:C], ltkv[2 * pair][:],
                                     AF.Copy)
                nc.scalar.activation(lb[:, :, C:128], ltkv[2 * pair + 1][:],
                                     AF.Copy)
                lb_tiles.append(lb)

            # Staging gate: a 1-element DMA into each emb_bf tensor whose
            # source depends on the LAST pair's cast.  The 1MB staging
            # writes (WAW on that element) and the XBAR transposes are
            # thereby deferred until the input loads have drained, so they
            # don't steal HBM bandwidth from the critical input stream.
            # Everything lives on the sync ring (emitted later, after the
            # stats) so the ACT ring never stalls behind a staging issue.
            gate_sb = smallp.tile([1, 4], BF, tag="gate")
            nc.vector.tensor_tensor(
                out=gate_sb[:], in0=lb_tiles[3][0:1, 0, 0:4],
                in1=lb_tiles[3][0:1, 0, 0:4], op=ALU.bypass,
            )
            etp_tiles = []

            def stage_etp():
                for pair in range(4):
                    nc.sync.dma_start(
                        emb_bfs[pair][0:1, 0:1], gate_sb[0:1, pair:pair + 1]
                    )
                for pair in range(4):
                    nc.sync.dma_start(
                        emb_bfs[pair].rearrange("(p nb) c -> p nb c", p=128),
                        lb_tiles[pair][:],
                    )
                for pair in range(4):
                    etp = streamp.tile([128, N], BF, tag="etp", bufs=4)
                    nc.sync.dma_start_transpose(etp[:], emb_bfs[pair][:])
                    etp_tiles.append(etp)

            # ===== persistent small tensors =====
            ssum = smallp.tile([128, 2, 4], dt.float32, tag="ssum")
            ssq = smallp.tile([128, 2, 4], dt.float32, tag="ssq")
            gmtmp = smallp.tile([128, CH], dt.float32, tag="gmtmp")
            mrs = smallp.tile([128, 1], dt.float32, tag="mrs", bufs=2)
            stats2 = smallp.tile([128, 8, 2], dt.float32, tag="stats2")
            invden2 = smallp.tile([128, 2, CB], dt.float32, tag="invden2")
            den_acc = [
                smallp.tile([128, CB, 1], dt.float32, tag="denacc", bufs=2,
                            name=f"den{i}")
                for i in range(2)
            ]

            m_tiles = {}

            def m_phase_pe(inst, pair):
                """E2T = emb_q^T @ [emb_2p | emb_2p+1]  ([64, 128])."""
                pE = psp.tile([128, 512], dt.float32, tag="pp",
                              name=f"pE{inst}_{pair}")
                lb = lb_tiles[pair]
                ebq = ebq_tiles[inst]
                for nb in range(NB):
                    nc.tensor.matmul(
                        pE[0:C, 0:128],
                        ebq[:, nb, :],
                        lb[:, nb, :],
                        start=(nb == 0),
                        stop=(nb == NB - 1),
                    )
                e2t = streamp.tile([C, 128], BF, tag="e2t")
                nc.vector.tensor_copy(out=e2t[:], in_=pE[0:C, 0:128])
                return e2t

            def m_phase_m(inst, pair, e2t):
                """m = E2T^T @ Wq ([128, CH]); plane-sum partials."""
                pm = psp.tile([128, 512], dt.float32, tag="pp",
                              name=f"pm{inst}_{pair}")
                nc.tensor.matmul(
                    pm[:], e2t[:], Wq_b[:], start=True, stop=True
                )
                m_sb = streamp.tile([128, CH], BF, tag="msb", bufs=8)
                nc.vector.tensor_copy(out=m_sb[:], in_=pm[:])
                m_tiles[(inst, pair)] = m_sb
                nc.vector.tensor_reduce(
                    out=mrs[:], in_=m_sb[:], axis=mybir.AxisListType.X,
                    op=ALU.add,
                )
                nc.vector.tensor_tensor(
                    out=ssum[:, inst, pair:pair + 1], in0=mrs[:],
                    in1=wk2s[:], op=ALU.mult,
                )

            def pair_ssq(inst, pair):
                """plane sum-of-squares partial: <m, G2m @ m>."""
                m_sb = m_tiles[(inst, pair)]
                pgm = psp.tile([128, 512], dt.float32, tag="pp",
                               name=f"pgm{inst}_{pair}")
                nc.tensor.matmul(
                    pgm[:], G2m_b[:], m_sb[:], start=True, stop=True
                )
                nc.vector.tensor_tensor(
                    out=gmtmp[:], in0=m_sb[:], in1=pgm[:], op=ALU.mult,
                )
                nc.vector.tensor_reduce(
                    out=ssq[:, inst, pair:pair + 1], in_=gmtmp[:],
                    axis=mybir.AxisListType.X, op=ALU.add,
                )

            def stats_prep_both():
                """InstanceNorm stats for BOTH instances in one pass.
                Everything on DVE except a single [128, 2] Sqrt, so the
                ACT engine sees at most one table switch before the exp
                stream (table thrash mid-stream costs 1.3us per switch).
                stats2 layout: [128, stat, inst]."""
                red = smallp.tile([128, 4], dt.float32, tag="red")
                for inst in range(2):
                    nc.vector.tensor_reduce(
                        out=red[:, 2 * inst:2 * inst + 1],
                        in_=ssum[:, inst, :],
                        axis=mybir.AxisListType.X, op=ALU.add,
                    )
                    nc.vector.tensor_reduce(
                        out=red[:, 2 * inst + 1:2 * inst + 2],
                        in_=ssq[:, inst, :],
                        axis=mybir.AxisListType.X, op=ALU.add,
                    )
                red_r = smallp.tile([128, 4], dt.float32r, tag="red_r")
                nc.vector.tensor_copy(out=red_r[:], in_=red[:])
                ptr = psp.tile([128, 512], dt.float32, tag="pp", name="ptr")
                nc.tensor.matmul(
                    ptr[:, 0:4], ones_r[:], red_r[:], start=True, stop=True
                )
                # ptr cols = (sum0, sumsq0, sum1, sumsq1) -> stat-major
                nc.vector.tensor_scalar_mul(
                    stats2[:, 0:2, :],
                    ptr[:, 0:4].rearrange("p (i k) -> p k i", i=2),
                    1.0 / PLANE,
                )
                mu = stats2[:, 0, :]
                ex2 = stats2[:, 1, :]
                musq = stats2[:, 2, :]
                var = stats2[:, 3, :]
                std = stats2[:, 4, :]
                rstd = stats2[:, 5, :]
                nmr = stats2[:, 6, :]
                nc.vector.tensor_tensor(out=musq, in0=mu, in1=mu, op=ALU.mult)
                nc.vector.tensor_tensor(out=var, in0=ex2, in1=musq,
                                        op=ALU.subtract)
                nc.vector.tensor_scalar_add(var, var, EPS)
                nc.scalar.activation(std, var, AF.Sqrt, bias=0.0)
                nc.vector.reciprocal(rstd, std)
                nc.vector.tensor_tensor(out=nmr, in0=mu, in1=rstd,
                                        op=ALU.mult)
                nc.vector.tensor_scalar_mul(nmr, nmr, -1.0)

            sT_tiles = [
                resp.tile([128, NB, CH], BF, tag="sT", bufs=2,
                          name=f"sT{i}")
                for i in range(2)
            ]

            def sT_fill(inst, db):
                """sT[d, c] = exp(rstd * Wk^T m - mu*rstd): two 2-bank PSUM
                chunks, each drained [128, 1024] by ACT with fused exp."""
                half = (db % 2) * C
                m_sb = m_tiles[(inst, db // 2)]
                sT = sT_tiles[inst]
                for c in range(2):
                    ps2 = psp.tile([128, 2, 512], dt.float32, tag="ps2",
                                   bufs=3, name=f"ps2_{inst}_{db}_{c}")
                    for d2 in range(2):
                        dc = 2 * c + d2
                        nc.tensor.matmul(
                            ps2[:, d2, :],
                            Wk2_b[half:half + C, dc * 128:(dc + 1) * 128],
                            m_sb[half:half + C, :],
                            start=True,
                            stop=True,
                        )
                    kb = db * CB + 2 * c
                    nc.scalar.activation(
                        sT[:, kb:kb + 2, :], ps2[:], AF.Exp,
                        bias=stats2[:, 6, inst:inst + 1],
                        scale=stats2[:, 5, inst:inst + 1],
                    )

            wva_tiles = [
                streamp.tile([128, B, CB, C], BF, tag="wva8", bufs=2,
                             name=f"wva{i}")
                for i in range(2)
            ]

            def wva(inst, db):
                """wva_db = (a_db Wv^T | den partials): 16 PE matmuls off
                the exp'd sT block, drained by a single DVE copy."""
                sT = sT_tiles[inst]
                W65 = C + 1
                pwa = psp.tile([128, 512], dt.float32, tag="pp",
                               name=f"pwa{inst}_{db}")
                for chb in range(CB):
                    for j in range(CB):
                        nc.tensor.matmul(
                            pwa[:, chb * 128:chb * 128 + W65],
                            sT[:, db * CB + j, chb * 128:(chb + 1) * 128],
                            WvT_b[:, j, :],
                            start=(j == 0),
                            stop=(j == CB - 1),
                        )
                wva_all = wva_tiles[inst]
                pview = pwa[:].rearrange("p (a b) -> p a b", b=128)
                nc.vector.tensor_copy(
                    out=wva_all[:, db, :, :], in_=pview[:, :, 0:C],
                )
                dcol = pview[:, :, C:C + 1]
                if db == 0:
                    nc.vector.tensor_copy(out=den_acc[inst][:], in_=dcol)
                else:
                    nc.vector.tensor_tensor(
                        out=den_acc[inst][:], in0=den_acc[inst][:],
                        in1=dcol, op=ALU.add,
                    )

            def ctx_out(inst):
                """outT = sum_db (wvo_db)^T @ emb_db^T with
                wvo_db = wva_db @ (Wout / den)."""
                wva_all = wva_tiles[inst]
                nc.vector.reciprocal(
                    invden2[:, inst, :], den_acc[inst][:, :, 0]
                )
                wout_s = streamp.tile([128, CB, C], BF, tag="wout_s",
                                      bufs=2)
                for k in range(CB):
                    nc.vector.tensor_scalar_mul(
                        wout_s[:, k, :], Wout_b[:, k, :],
                        invden2[:, inst, k:k + 1],
                    )
                pwo = psp.tile([128, 512], dt.float32, tag="pp",
                               name=f"pwo{inst}")
                for db in range(B):
                    for chb in range(CB):
                        nc.tensor.matmul(
                            pwo[0:C, db * C:(db + 1) * C],
                            wva_all[:, db, chb, :],
                            wout_s[:, chb, :],
                            start=(chb == 0),
                            stop=(chb == CB - 1),
                        )
                # pair-stack wvo: [128(2 batches' c'), 4(pair), 64]
                wvo2 = streamp.tile([128, CB, C], BF, tag="wvo2", bufs=2)
                for db in range(B):
                    nc.vector.tensor_copy(
                        out=wvo2[(db % 2) * C:(db % 2 + 1) * C, db // 2, :],
                        in_=pwo[0:C, db * C:(db + 1) * C],
                    )
                # outT[c, n] = sum_pairs wvo_pair^T @ embT_pair
                for g in range(8):
                    pout = psp.tile([128, 512], dt.float32, tag="pp",
                                    name=f"pout{inst}_{g}")
                    for k in range(4):
                        nc.tensor.matmul(
                            pout[0:C, :],
                            wvo2[:, k, :],
                            etp_tiles[k][:, g * 512:(g + 1) * 512],
                            start=(k == 0),
                            stop=(k == 3),
                        )
                    ot = streamp.tile([C, 512], dt.float32, tag="ot",
                                      bufs=2)
                    # DVE-only drains: any ACT op here would clock-gate
                    # the PE (K=4) during the pure-matmul endgame.
                    nc.vector.tensor_copy(out=ot[:], in_=pout[0:C, :])
                    nc.sync.dma_start(
                        out_d[inst, :, g * 512:(g + 1) * 512], ot[:]
                    )

            # ================= schedule =================
            # m phases chase the input stream: dense PE chains per pair
            # (both instances); ssq partials overlap the load window.
            for pair in range(4):
                e2ts = [m_phase_pe(inst, pair) for inst in range(2)]
                for inst in range(2):
                    m_phase_m(inst, pair, e2ts[inst])
                for inst in range(2):
                    pair_ssq(inst, pair)
            stats_prep_both()
            stage_etp()

            # middle: ACT-bound exp-drain stream, fills only.
            for db in range(B):
                sT_fill(0, db)
            for db in range(B):
                sT_fill(1, db)
            # Gate inst1's wva matmuls behind the LAST exp drain (bypass
            # write into a corner each wva reads): during the exp stream
            # the PE is clock-gated to 1.2GHz, so bulk matmul work there
            # costs double.  inst0's wva is left free: the fills leave
            # ~8us of PE slack under the ACT stream, and finishing wva(0)
            # early lets the den->pwo->outT chain for inst0 start at the
            # first moment the exps end instead of serializing after it.
            nc.vector.tensor_tensor(
                out=sT_tiles[1][0:1, 0, 0:1],
                in0=sT_tiles[1][0:1, 0, 0:1],
                in1=sT_tiles[1][0:1, NB - 1, CH - 1:CH],
                op=ALU.bypass,
            )
            for db in range(B):
                wva(0, db)
                wva(1, db)

            ctx_out(0)
            ctx_out(1)

    nc.compile()
    return nc


def _get_nc():
    global _nc
    if _nc is None:
        _nc = _build()
    return _nc


def kernel(emb, Wq, Wk, Wv, Wout):
    emb = np.ascontiguousarray(emb, dtype=np.float32)
    Wq = np.ascontiguousarray(Wq, dtype=np.float32)
    Wk = np.ascontiguousarray(Wk, dtype=np.float32)
    Wv = np.ascontiguousarray(Wv, dtype=np.float32)
    Wout = np.ascontiguousarray(Wout, dtype=np.float32)
    emb_l, emb_u = emb[:B], emb[B:]
    ident = np.eye(128, dtype=np.float32)
    ones = np.ones((128, 128), dtype=np.float32)

    in_maps = []
    for core in range(8):
        if core < 4:
            qb, kvb = emb_l[2 * core:2 * core + 2], emb_u
        else:
            j = core - 4
            qb, kvb = emb_u[2 * j:2 * j + 2], emb_l
        in_maps.append({
            "embq": np.ascontiguousarray(qb), "embkv": np.ascontiguousarray(kvb),
            "Wq": Wq, "Wk": Wk, "Wv": Wv, "Wout": Wout, "ident": ident,
            "ones": ones,
        })

    res = run_bass_kernel_spmd(_get_nc(), in_maps, list(range(8))).results

    out = np.empty((2 * B, N, C), np.float32)
    for core in range(8):
        o = res[core]["out"].transpose(0, 2, 1)  # [2, C, N] -> [2, N, C]
        if core < 4:
            out[2 * core:2 * core + 2] = o
        else:
            j = core - 4
            out[B + 2 * j:B + 2 * j + 2] = o
    return out
